# revision 1
# baseline (speedup 1.0000x reference)
"""GCN decoder (2x GCNConv + linear) on 8 Trainium2 NeuronCores.

Math (from the reference):
  adj = sigmoid(z z^T) values are discarded; only new_edge_index (a static
  full row-major grid of indices) survives. The two GCN convs run on the
  ORIGINAL sparse edges + self-loops:
      conv(x) = relu(D^-1/2 (A_w + I) D^-1/2 (x @ W) + b)
  then out = h @ lin_W + lin_b.

Strategy:
  Densify the normalized adjacency on the host:
      Bt[s, d] = s[s] * (A_w + I)^T[s, d] * s[d],   s = deg^-1/2
  so each conv aggregation is a dense matmul y^T = u^T_stat @ Bt (node
  contraction on the PE partition axis). Activations are kept in transposed
  [feature, node] layout on the device, which makes every x @ W step
  consume the previous layer's output directly as the stationary operand —
  no on-device transposes anywhere.

  Sharding: column-shard Bt across the 8 cores (each core owns 1024
  destination nodes); replicate z and the small weights; AllGather the
  [256, 1024] f16 hidden block between the two convs. Adjacency is cast to
  f16 (values are O(0.1) positives; f16 keeps ~5e-4 relative error) which
  halves the dominant HBM traffic; all matmuls accumulate in f32 PSUM.
"""

from contextlib import ExitStack

import numpy as np

import concourse.bass as bass
import concourse.tile as tile
from concourse import bacc, mybir
from concourse.bass_utils import run_bass_kernel_spmd

N = 8192  # nodes
ZF = 128  # z features
H = 256  # GCN hidden width
XF = 128  # output features
N_CORES = 8
P = N // N_CORES  # 1024 destination nodes per core
SC = N // 128  # 64 source-node chunks
F16 = mybir.dt.float16
F32 = mybir.dt.float32

# Set by a test harness to capture an NTFF profile of the SPMD run.
TRACE = False
LAST_RESULT = None

_STATE = {}


def _build_module():
    nc = bacc.Bacc("TRN2", target_bir_lowering=False, debug=False, num_devices=N_CORES)

    zT = nc.dram_tensor("zT", [ZF, N], F16, kind="ExternalInput").ap()
    Bt = nc.dram_tensor("Bt", [N, P], F16, kind="ExternalInput").ap()
    W0f = nc.dram_tensor("W0f", [ZF, H], F16, kind="ExternalInput").ap()
    W1f = nc.dram_tensor("W1f", [2, 128, H], F16, kind="ExternalInput").ap()
    LWf = nc.dram_tensor("LWf", [2, 128, XF], F16, kind="ExternalInput").ap()
    b0c = nc.dram_tensor("b0c", [2, 128, 1], F32, kind="ExternalInput").ap()
    b1c = nc.dram_tensor("b1c", [2, 128, 1], F32, kind="ExternalInput").ap()
    lbb = nc.dram_tensor("lbb", [128, XF], F32, kind="ExternalInput").ap()
    OUT = nc.dram_tensor("OUT", [P, XF], F32, kind="ExternalOutput").ap()

    relu = mybir.ActivationFunctionType.Relu

    with tile.TileContext(nc) as tc:
        with ExitStack() as ctx:
            const = ctx.enter_context(tc.tile_pool(name="const", bufs=1))
            acts = ctx.enter_context(tc.tile_pool(name="acts", bufs=1))
            bpool = ctx.enter_context(tc.tile_pool(name="bpool", bufs=4))
            psa = ctx.enter_context(tc.tile_pool(name="psa", bufs=4, space="PSUM"))
            psb = ctx.enter_context(tc.tile_pool(name="psb", bufs=4, space="PSUM"))
            dram = ctx.enter_context(tc.tile_pool(name="dram", bufs=1, space="DRAM"))

            # ---- constants ----
            zT_sb = const.tile([128, N], F16)
            nc.sync.dma_start(zT_sb[:], zT[:])
            W0_sb = const.tile([128, H], F16)
            nc.sync.dma_start(W0_sb[:], W0f[:])
            W1_sb = [const.tile([128, H], F16, name=f"W1_sb{i}") for i in range(2)]
            LW_sb = [const.tile([128, XF], F16, name=f"LW_sb{i}") for i in range(2)]
            b0_sb = [const.tile([128, 1], F32, name=f"b0_sb{i}") for i in range(2)]
            b1_sb = [const.tile([128, 1], F32, name=f"b1_sb{i}") for i in range(2)]
            for i in range(2):
                nc.sync.dma_start(W1_sb[i][:], W1f[i])
                nc.sync.dma_start(LW_sb[i][:], LWf[i])
                nc.sync.dma_start(b0_sb[i][:], b0c[i])
                nc.sync.dma_start(b1_sb[i][:], b1c[i])
            lbb_sb = const.tile([128, XF], F32)
            nc.sync.dma_start(lbb_sb[:], lbb[:])

            # collective buffers (must be internal DRAM; output Shared)
            cin_d = dram.tile([H, P], F16)
            cout_d = dram.tile([N_CORES * H, P], F16, addr_space="Shared")

            # ---- phase A1: u0 = z @ W0, kept as f16 [node-part, h-free] ----
            u0 = []
            for i in range(SC):
                pa = psa.tile([128, H], F32, tag="pa", name=f"pa1_{i}")
                nc.tensor.matmul(
                    pa[:],
                    zT_sb[:, i * 128 : (i + 1) * 128],
                    W0_sb[:],
                    start=True,
                    stop=True,
                )
                u = acts.tile([128, H], F16, tag="u0", bufs=SC, name=f"u0_{i}")
                nc.vector.tensor_copy(u[:], pa[:])
                u0.append(u)

            # ---- phase B1: y1^T[h, d] = sum_s u0[s, h] * Bt[s, d] ----
            ps1 = [
                [
                    psb.tile([128, 512], F32, tag="pb", name=f"pb1_{hc}_{dc}")
                    for dc in range(2)
                ]
                for hc in range(2)
            ]
            for sc in range(SC):
                bt = bpool.tile([128, P], F16, tag="bt", name=f"bt1_{sc}")
                nc.sync.dma_start(bt[:], Bt[sc * 128 : (sc + 1) * 128, :])
                for hc in range(2):
                    for dc in range(2):
                        nc.tensor.matmul(
                            ps1[hc][dc][:],
                            u0[sc][:, hc * 128 : (hc + 1) * 128],
                            bt[:, dc * 512 : (dc + 1) * 512],
                            start=(sc == 0),
                            stop=(sc == SC - 1),
                        )

            # h1^T = relu(y1^T + b0)  -> DRAM -> AllGather
            h1T = [acts.tile([128, P], F16, tag="h1", bufs=2, name=f"h1_{hc}") for hc in range(2)]
            for hc in range(2):
                for dc in range(2):
                    nc.scalar.activation(
                        h1T[hc][:, dc * 512 : (dc + 1) * 512],
                        ps1[hc][dc][:],
                        relu,
                        bias=b0_sb[hc][:],
                        scale=1.0,
                    )
                nc.sync.dma_start(cin_d[hc * 128 : (hc + 1) * 128, :], h1T[hc][:])

            nc.gpsimd.collective_compute(
                "AllGather",
                mybir.AluOpType.bypass,
                ins=[cin_d.opt()],
                outs=[cout_d.opt()],
                replica_groups=[list(range(N_CORES))],
            )

            # ---- phase A2: u1 = h1 @ W1 ----
            hg = []
            for r in range(N_CORES):
                for hc in range(2):
                    g = acts.tile(
                        [128, P], F16, tag="hg", bufs=2 * N_CORES, name=f"hg_{r}_{hc}"
                    )
                    nc.sync.dma_start(
                        g[:],
                        cout_d[r * H + hc * 128 : r * H + (hc + 1) * 128, :],
                    )
                    hg.append(g)
            u1 = []
            for i in range(SC):
                r, lc = i // N_CORES, i % N_CORES
                pa = psa.tile([128, H], F32, tag="pa", name=f"pa2_{i}")
                for hc in range(2):
                    nc.tensor.matmul(
                        pa[:],
                        hg[r * 2 + hc][:, lc * 128 : (lc + 1) * 128],
                        W1_sb[hc][:],
                        start=(hc == 0),
                        stop=(hc == 1),
                    )
                u = acts.tile([128, H], F16, tag="u1", bufs=SC, name=f"u1_{i}")
                nc.vector.tensor_copy(u[:], pa[:])
                u1.append(u)

            # ---- phase B2 ----
            ps2 = [
                [
                    psb.tile([128, 512], F32, tag="pb", name=f"pb2_{hc}_{dc}")
                    for dc in range(2)
                ]
                for hc in range(2)
            ]
            for sc in range(SC):
                bt = bpool.tile([128, P], F16, tag="bt", name=f"bt2_{sc}")
                nc.sync.dma_start(bt[:], Bt[sc * 128 : (sc + 1) * 128, :])
                for hc in range(2):
                    for dc in range(2):
                        nc.tensor.matmul(
                            ps2[hc][dc][:],
                            u1[sc][:, hc * 128 : (hc + 1) * 128],
                            bt[:, dc * 512 : (dc + 1) * 512],
                            start=(sc == 0),
                            stop=(sc == SC - 1),
                        )

            h2T = [acts.tile([128, P], F16, tag="h2", bufs=2, name=f"h2_{hc}") for hc in range(2)]
            for hc in range(2):
                for dc in range(2):
                    nc.scalar.activation(
                        h2T[hc][:, dc * 512 : (dc + 1) * 512],
                        ps2[hc][dc][:],
                        relu,
                        bias=b1_sb[hc][:],
                        scale=1.0,
                    )

            # ---- final linear: out[n, o] = h2[n, :] @ lin_W + lin_b ----
            for lc in range(N_CORES):
                pf = psa.tile([128, XF], F32, tag="pa", name=f"pf_{lc}")
                for gc in range(2):
                    nc.tensor.matmul(
                        pf[:],
                        h2T[gc][:, lc * 128 : (lc + 1) * 128],
                        LW_sb[gc][:],
                        start=(gc == 0),
                        stop=(gc == 1),
                    )
                ot = acts.tile([128, XF], F32, tag="ot", bufs=2, name=f"ot_{lc}")
                nc.vector.tensor_add(ot[:], pf[:], lbb_sb[:])
                nc.sync.dma_start(OUT[lc * 128 : (lc + 1) * 128, :], ot[:])

    nc.compile()
    return nc


def _get_module():
    if "nc" not in _STATE:
        _STATE["nc"] = _build_module()
    return _STATE["nc"]


def kernel(z_, edge_index, edge_attr, W0, b0, W1, b1, lin_W, lin_b):
    global LAST_RESULT
    z_ = np.asarray(z_, dtype=np.float32)
    edge_index = np.asarray(edge_index)
    edge_attr = np.asarray(edge_attr, dtype=np.float32)
    W0 = np.asarray(W0, dtype=np.float32)
    b0 = np.asarray(b0, dtype=np.float32)
    W1 = np.asarray(W1, dtype=np.float32)
    b1 = np.asarray(b1, dtype=np.float32)
    lin_W = np.asarray(lin_W, dtype=np.float32)
    lin_b = np.asarray(lin_b, dtype=np.float32)

    src = edge_index[0].astype(np.int64)
    dst = edge_index[1].astype(np.int64)

    # deg = segment_sum(w, dst) over edges + unit self-loops
    deg = np.zeros(N, np.float32)
    np.add.at(deg, dst, edge_attr)
    deg += 1.0
    s = (1.0 / np.sqrt(deg)).astype(np.float32)

    # Bt[s, d] = s[s] * (A_w + I)^T * s[d]
    Bm = np.zeros((N, N), np.float32)
    np.add.at(Bm, (src, dst), edge_attr)
    Bm[np.arange(N), np.arange(N)] += 1.0
    Bm *= s[:, None]
    Bm *= s[None, :]
    Bh = Bm.astype(np.float16)
    del Bm

    zT16 = np.ascontiguousarray(z_.T).astype(np.float16)
    common = {
        "zT": zT16,
        "W0f": W0.astype(np.float16),
        "W1f": np.ascontiguousarray(W1.reshape(2, 128, H)).astype(np.float16),
        "LWf": np.ascontiguousarray(lin_W.reshape(2, 128, XF)).astype(np.float16),
        "b0c": np.ascontiguousarray(b0.reshape(2, 128, 1)),
        "b1c": np.ascontiguousarray(b1.reshape(2, 128, 1)),
        "lbb": np.ascontiguousarray(np.broadcast_to(lin_b, (128, XF))),
    }
    in_maps = [
        dict(common, Bt=np.ascontiguousarray(Bh[:, k * P : (k + 1) * P]))
        for k in range(N_CORES)
    ]

    nc = _get_module()
    res = run_bass_kernel_spmd(
        nc, in_maps, core_ids=list(range(N_CORES)), trace=TRACE
    )
    LAST_RESULT = res
    out = np.concatenate([res.results[k]["OUT"] for k in range(N_CORES)], axis=0)

    # new_edge_index: static full row-major grid, like the reference's
    # repeat/tile of arange(n) (int32 under default jax, int64 under x64).
    idt = np.int64 if edge_index.dtype == np.int64 else np.int32
    nei = np.empty((2, N * N), dtype=idt)
    nei[0] = np.repeat(np.arange(N, dtype=idt), N)
    nei[1] = np.tile(np.arange(N, dtype=idt), N)
    return out, nei


# revision 2
# speedup vs baseline: 1.0330x; 1.0330x over previous
"""GCN decoder (2x GCNConv + linear) on 8 Trainium2 NeuronCores.

Math (from the reference):
  adj = sigmoid(z z^T) values are discarded; only new_edge_index (a static
  full row-major grid of indices) survives. The two GCN convs run on the
  ORIGINAL sparse edges + self-loops:
      conv(x) = relu(D^-1/2 (A_w + I) D^-1/2 (x @ W) + b)
  then out = h @ lin_W + lin_b.

Strategy:
  Densify the normalized adjacency on the host:
      Bt[s, d] = s[s] * (A_w + I)^T[s, d] * s[d],   s = deg^-1/2
  so each conv aggregation is a dense matmul y^T = u_stat @ Bt (node
  contraction on the PE partition axis). Aggregation outputs live in
  transposed [feature, node] layout, which each following x @ W step
  consumes directly as the stationary operand — no on-device transposes.

  Sharding: column-shard Bt across the 8 cores (each core owns 1024
  destination nodes); replicate z and the small weights. Between the convs
  each core computes u1 = h1 @ W1 for ITS OWN nodes only (u1 is row-wise,
  so it needs no remote data), and u1 is AllGathered in 4 chunks; conv2's
  accumulation consumes the chunks in arrival order, hiding the collective
  behind matmuls. The dc=0 half of Bt stays resident in SBUF after conv1
  so conv2 only streams the other half. Adjacency and activations are f16
  (~5e-4 relative error) halving HBM traffic; PSUM accumulates in f32.
"""

from contextlib import ExitStack

import numpy as np

import concourse.bass as bass
import concourse.tile as tile
from concourse import bacc, mybir
from concourse.bass_utils import run_bass_kernel_spmd

N = 8192  # nodes
ZF = 128  # z features
H = 256  # GCN hidden width
XF = 128  # output features
N_CORES = 8
P = N // N_CORES  # 1024 destination nodes per core
SC = N // 128  # 64 source-node chunks
NL = P // 128  # 8 local node chunks
F16 = mybir.dt.float16
F32 = mybir.dt.float32

# Set by a test harness to capture an NTFF profile of the SPMD run.
TRACE = False
LAST_RESULT = None

_STATE = {}


def _build_module():
    nc = bacc.Bacc("TRN2", target_bir_lowering=False, debug=False, num_devices=N_CORES)

    zT = nc.dram_tensor("zT", [ZF, N], F16, kind="ExternalInput").ap()
    Bt = nc.dram_tensor("Bt", [N, P], F16, kind="ExternalInput").ap()
    W0f = nc.dram_tensor("W0f", [ZF, H], F16, kind="ExternalInput").ap()
    W1f = nc.dram_tensor("W1f", [2, 128, H], F16, kind="ExternalInput").ap()
    LWf = nc.dram_tensor("LWf", [2, 128, XF], F16, kind="ExternalInput").ap()
    b0c = nc.dram_tensor("b0c", [2, 128, 1], F32, kind="ExternalInput").ap()
    b1c = nc.dram_tensor("b1c", [2, 128, 1], F32, kind="ExternalInput").ap()
    lbb = nc.dram_tensor("lbb", [128, XF], F32, kind="ExternalInput").ap()
    OUT = nc.dram_tensor("OUT", [P, XF], F32, kind="ExternalOutput").ap()

    relu = mybir.ActivationFunctionType.Relu

    with tile.TileContext(nc) as tc:
        with ExitStack() as ctx:
            const = ctx.enter_context(tc.tile_pool(name="const", bufs=1))
            acts = ctx.enter_context(tc.tile_pool(name="acts", bufs=1))
            bres = ctx.enter_context(tc.tile_pool(name="bres", bufs=1))
            bstr = ctx.enter_context(tc.tile_pool(name="bstr", bufs=6))
            psa = ctx.enter_context(tc.tile_pool(name="psa", bufs=4, space="PSUM"))
            psb = ctx.enter_context(tc.tile_pool(name="psb", bufs=4, space="PSUM"))
            dram = ctx.enter_context(tc.tile_pool(name="dram", bufs=1, space="DRAM"))

            # ---- constants (zT in chunks so A1 starts after the first one) ----
            zTc = []
            for c in range(NL):
                zt = const.tile([128, P], F16, name=f"zT_sb{c}")
                nc.sync.dma_start(zt[:], zT[:, c * P : (c + 1) * P])
                zTc.append(zt)
            W0_sb = const.tile([128, H], F16)
            nc.sync.dma_start(W0_sb[:], W0f[:])
            W1_sb = [const.tile([128, H], F16, name=f"W1_sb{i}") for i in range(2)]
            LW_sb = [const.tile([128, XF], F16, name=f"LW_sb{i}") for i in range(2)]
            b0_sb = [const.tile([128, 1], F32, name=f"b0_sb{i}") for i in range(2)]
            b1_sb = [const.tile([128, 1], F32, name=f"b1_sb{i}") for i in range(2)]
            for i in range(2):
                nc.sync.dma_start(W1_sb[i][:], W1f[i])
                nc.sync.dma_start(LW_sb[i][:], LWf[i])
                nc.sync.dma_start(b0_sb[i][:], b0c[i])
                nc.sync.dma_start(b1_sb[i][:], b1c[i])
            lbb_sb = const.tile([128, XF], F32)
            nc.sync.dma_start(lbb_sb[:], lbb[:])

            # collective buffers: u1 allgathered in 4 chunks of 2 node-blocks
            cin = [dram.tile([H, H], F16, name=f"cin{j}") for j in range(4)]
            cout = [
                dram.tile([N_CORES * H, H], F16, addr_space="Shared", name=f"cout{j}")
                for j in range(4)
            ]

            # ---- A1: u0 = z @ W0 as f16 [node-part, h-free] ----
            u0 = []
            for i in range(SC):
                pa = psa.tile([128, H], F32, tag="pa", name=f"pa1_{i}")
                nc.tensor.matmul(
                    pa[:],
                    zTc[i // NL][:, (i % NL) * 128 : (i % NL + 1) * 128],
                    W0_sb[:],
                    start=True,
                    stop=True,
                )
                u = acts.tile([128, H], F16, tag="u0", bufs=SC, name=f"u0_{i}")
                nc.vector.tensor_copy(u[:], pa[:])
                u0.append(u)

            # ---- B1: y1^T[h, d] = sum_s u0[s, h] * Bt[s, d] ----
            # dc=0 halves of Bt stay resident for conv2; dc=1 halves stream.
            bt0 = []
            ps1 = [
                [
                    psb.tile([128, 512], F32, tag="pb", name=f"pb1_{hc}_{dc}")
                    for dc in range(2)
                ]
                for hc in range(2)
            ]
            for sc in range(SC):
                b0t = bres.tile([128, 512], F16, tag="bt0", bufs=SC, name=f"bt0_{sc}")
                nc.sync.dma_start(b0t[:], Bt[sc * 128 : (sc + 1) * 128, 0:512])
                bt0.append(b0t)
                b1t = bstr.tile([128, 512], F16, tag="bts", name=f"bt1_{sc}")
                nc.gpsimd.dma_start(b1t[:], Bt[sc * 128 : (sc + 1) * 128, 512:1024])
                for hc in range(2):
                    nc.tensor.matmul(
                        ps1[hc][0][:],
                        u0[sc][:, hc * 128 : (hc + 1) * 128],
                        b0t[:],
                        start=(sc == 0),
                        stop=(sc == SC - 1),
                    )
                    nc.tensor.matmul(
                        ps1[hc][1][:],
                        u0[sc][:, hc * 128 : (hc + 1) * 128],
                        b1t[:],
                        start=(sc == 0),
                        stop=(sc == SC - 1),
                    )

            # h1^T = relu(y1^T + b0), per (hc, dc) quarter
            h1 = [
                [
                    acts.tile([128, 512], F16, tag="h1", bufs=4, name=f"h1_{hc}_{dc}")
                    for dc in range(2)
                ]
                for hc in range(2)
            ]
            for hc in range(2):
                for dc in range(2):
                    nc.scalar.activation(
                        h1[hc][dc][:],
                        ps1[hc][dc][:],
                        relu,
                        bias=b0_sb[hc][:],
                        scale=1.0,
                    )

            # ---- A2 (local rows only) + chunked AllGather of u1 ----
            # u1[n, g] = sum_h h1[n, h] W1[h, g] for this core's 1024 nodes.
            for jp in range(4):
                for j in (2 * jp, 2 * jp + 1):
                    pa = psa.tile([128, H], F32, tag="pa", name=f"pa2_{j}")
                    for hc in range(2):
                        nc.tensor.matmul(
                            pa[:],
                            h1[hc][j // 4][:, (j % 4) * 128 : (j % 4 + 1) * 128],
                            W1_sb[hc][:],
                            start=(hc == 0),
                            stop=(hc == 1),
                        )
                    u = acts.tile([128, H], F16, tag="u1", bufs=NL, name=f"u1_{j}")
                    nc.vector.tensor_copy(u[:], pa[:])
                    nc.sync.dma_start(
                        cin[jp][(j % 2) * 128 : (j % 2 + 1) * 128, :], u[:]
                    )
                nc.gpsimd.collective_compute(
                    "AllGather",
                    mybir.AluOpType.bypass,
                    ins=[cin[jp].opt()],
                    outs=[cout[jp].opt()],
                    replica_groups=[list(range(N_CORES))],
                )

            # ---- B2: y2^T[g, d] = sum_s u1[s, g] * Bt[s, d] ----
            # consume allgathered u1 chunks in arrival order
            ps2 = [
                [
                    psb.tile([128, 512], F32, tag="pb", name=f"pb2_{gc}_{dc}")
                    for dc in range(2)
                ]
                for gc in range(2)
            ]
            first = True
            n_done = 0
            for jp in range(4):
                for r in range(N_CORES):
                    for j in (2 * jp, 2 * jp + 1):
                        sc = r * NL + j
                        ug = acts.tile(
                            [128, H], F16, tag="ug", bufs=8, name=f"ug_{sc}"
                        )
                        nc.gpsimd.dma_start(
                            ug[:],
                            cout[jp][
                                r * H + (j % 2) * 128 : r * H + (j % 2 + 1) * 128, :
                            ],
                        )
                        b1t = bstr.tile([128, 512], F16, tag="bts", name=f"bt2_{sc}")
                        nc.sync.dma_start(
                            b1t[:], Bt[sc * 128 : (sc + 1) * 128, 512:1024]
                        )
                        n_done += 1
                        last = n_done == SC
                        for gc in range(2):
                            nc.tensor.matmul(
                                ps2[gc][0][:],
                                ug[:, gc * 128 : (gc + 1) * 128],
                                bt0[sc][:],
                                start=first,
                                stop=last,
                            )
                            nc.tensor.matmul(
                                ps2[gc][1][:],
                                ug[:, gc * 128 : (gc + 1) * 128],
                                b1t[:],
                                start=first,
                                stop=last,
                            )
                        first = False

            h2 = [
                [
                    acts.tile([128, 512], F16, tag="h2", bufs=4, name=f"h2_{gc}_{dc}")
                    for dc in range(2)
                ]
                for gc in range(2)
            ]
            for gc in range(2):
                for dc in range(2):
                    nc.scalar.activation(
                        h2[gc][dc][:],
                        ps2[gc][dc][:],
                        relu,
                        bias=b1_sb[gc][:],
                        scale=1.0,
                    )

            # ---- final linear: out[n, o] = h2[n, :] @ lin_W + lin_b ----
            for lc in range(NL):
                pf = psa.tile([128, XF], F32, tag="pa", name=f"pf_{lc}")
                for gc in range(2):
                    nc.tensor.matmul(
                        pf[:],
                        h2[gc][lc // 4][:, (lc % 4) * 128 : (lc % 4 + 1) * 128],
                        LW_sb[gc][:],
                        start=(gc == 0),
                        stop=(gc == 1),
                    )
                ot = acts.tile([128, XF], F32, tag="ot", bufs=2, name=f"ot_{lc}")
                nc.vector.tensor_add(ot[:], pf[:], lbb_sb[:])
                nc.sync.dma_start(OUT[lc * 128 : (lc + 1) * 128, :], ot[:])

    nc.compile()
    return nc


def _get_module():
    if "nc" not in _STATE:
        _STATE["nc"] = _build_module()
    return _STATE["nc"]


def kernel(z_, edge_index, edge_attr, W0, b0, W1, b1, lin_W, lin_b):
    global LAST_RESULT
    z_ = np.asarray(z_, dtype=np.float32)
    edge_index = np.asarray(edge_index)
    edge_attr = np.asarray(edge_attr, dtype=np.float32)
    W0 = np.asarray(W0, dtype=np.float32)
    b0 = np.asarray(b0, dtype=np.float32)
    W1 = np.asarray(W1, dtype=np.float32)
    b1 = np.asarray(b1, dtype=np.float32)
    lin_W = np.asarray(lin_W, dtype=np.float32)
    lin_b = np.asarray(lin_b, dtype=np.float32)

    src = edge_index[0].astype(np.int64)
    dst = edge_index[1].astype(np.int64)

    # deg = segment_sum(w, dst) over edges + unit self-loops
    deg = np.zeros(N, np.float32)
    np.add.at(deg, dst, edge_attr)
    deg += 1.0
    s = (1.0 / np.sqrt(deg)).astype(np.float32)

    # Bt[s, d] = s[s] * (A_w + I)^T * s[d]
    Bm = np.zeros((N, N), np.float32)
    np.add.at(Bm, (src, dst), edge_attr)
    Bm[np.arange(N), np.arange(N)] += 1.0
    Bm *= s[:, None]
    Bm *= s[None, :]
    Bh = Bm.astype(np.float16)
    del Bm

    zT16 = np.ascontiguousarray(z_.T).astype(np.float16)
    common = {
        "zT": zT16,
        "W0f": W0.astype(np.float16),
        "W1f": np.ascontiguousarray(W1.reshape(2, 128, H)).astype(np.float16),
        "LWf": np.ascontiguousarray(lin_W.reshape(2, 128, XF)).astype(np.float16),
        "b0c": np.ascontiguousarray(b0.reshape(2, 128, 1)),
        "b1c": np.ascontiguousarray(b1.reshape(2, 128, 1)),
        "lbb": np.ascontiguousarray(np.broadcast_to(lin_b, (128, XF))),
    }
    in_maps = [
        dict(common, Bt=np.ascontiguousarray(Bh[:, k * P : (k + 1) * P]))
        for k in range(N_CORES)
    ]

    nc = _get_module()
    res = run_bass_kernel_spmd(
        nc, in_maps, core_ids=list(range(N_CORES)), trace=TRACE
    )
    LAST_RESULT = res
    out = np.concatenate([res.results[k]["OUT"] for k in range(N_CORES)], axis=0)

    # new_edge_index: static full row-major grid, like the reference's
    # repeat/tile of arange(n) (int32 under default jax, int64 under x64).
    idt = np.int64 if edge_index.dtype == np.int64 else np.int32
    nei = np.empty((2, N * N), dtype=idt)
    nei[0] = np.repeat(np.arange(N, dtype=idt), N)
    nei[1] = np.tile(np.arange(N, dtype=idt), N)
    return out, nei


# revision 5
# speedup vs baseline: 1.1273x; 1.0913x over previous
"""GCN decoder (2x GCNConv + linear) on 8 Trainium2 NeuronCores.

Math (from the reference):
  adj = sigmoid(z z^T) values are discarded; only new_edge_index (a static
  full row-major grid of indices) survives. The two GCN convs run on the
  ORIGINAL sparse edges + self-loops:
      conv(x) = relu(D^-1/2 (A_w + I) D^-1/2 (x @ W) + b)
  then out = h @ lin_W + lin_b.

Strategy:
  Densify the normalized adjacency on the host:
      Bt[s, d] = s[s] * (A_w + I)^T[s, d] * s[d],   s = deg^-1/2
  so each conv aggregation is a dense matmul y^T = u_stat @ Bt (node
  contraction on the PE partition axis). Aggregation outputs live in
  transposed [feature, node] layout, which each following x @ W step
  consumes directly as the stationary operand — no on-device transposes.

  Sharding: column-shard Bt across the 8 cores (each core owns 1024
  destination nodes); replicate z and the small weights. Between the convs
  each core computes u1 = h1 @ W1 for ITS OWN nodes only (u1 is row-wise,
  so it needs no remote data), and u1 is AllGathered in 4 chunks; conv2's
  accumulation consumes the chunks in arrival order, hiding the collective
  behind matmuls. The dc=0 half of Bt stays resident in SBUF after conv1
  so conv2 only streams the other half. Adjacency and activations are f16
  (~5e-4 relative error) halving HBM traffic; PSUM accumulates in f32.
"""

from contextlib import ExitStack

import numpy as np

import concourse.bass as bass
import concourse.tile as tile
from concourse import bacc, mybir
from concourse.bass_utils import run_bass_kernel_spmd

N = 8192  # nodes
ZF = 128  # z features
H = 256  # GCN hidden width
XF = 128  # output features
N_CORES = 8
P = N // N_CORES  # 1024 destination nodes per core
SC = N // 128  # 64 source-node chunks
NL = P // 128  # 8 local node chunks
F16 = mybir.dt.float16
F32 = mybir.dt.float32

# Set by a test harness to capture an NTFF profile of the SPMD run.
TRACE = False
LAST_RESULT = None

_STATE = {}


def _build_module():
    nc = bacc.Bacc("TRN2", target_bir_lowering=False, debug=False, num_devices=N_CORES)

    zT = nc.dram_tensor("zT", [ZF, N], F16, kind="ExternalInput").ap()
    Bt = nc.dram_tensor("Bt", [N, P], F16, kind="ExternalInput").ap()
    W0f = nc.dram_tensor("W0f", [ZF, H], F16, kind="ExternalInput").ap()
    W1f = nc.dram_tensor("W1f", [2, 128, H], F16, kind="ExternalInput").ap()
    LWf = nc.dram_tensor("LWf", [2, 128, XF], F16, kind="ExternalInput").ap()
    b0c = nc.dram_tensor("b0c", [2, 128, 1], F32, kind="ExternalInput").ap()
    b1c = nc.dram_tensor("b1c", [2, 128, 1], F32, kind="ExternalInput").ap()
    lbb = nc.dram_tensor("lbb", [128, XF], F32, kind="ExternalInput").ap()
    OUT = nc.dram_tensor("OUT", [P, XF], F32, kind="ExternalOutput").ap()

    relu = mybir.ActivationFunctionType.Relu

    with tile.TileContext(nc) as tc:
        with ExitStack() as ctx:
            const = ctx.enter_context(tc.tile_pool(name="const", bufs=1))
            acts = ctx.enter_context(tc.tile_pool(name="acts", bufs=1))
            bres = ctx.enter_context(tc.tile_pool(name="bres", bufs=1))
            bstr = ctx.enter_context(tc.tile_pool(name="bstr", bufs=16))
            psa = ctx.enter_context(tc.tile_pool(name="psa", bufs=4, space="PSUM"))
            psb = ctx.enter_context(tc.tile_pool(name="psb", bufs=4, space="PSUM"))
            dram = ctx.enter_context(tc.tile_pool(name="dram", bufs=1, space="DRAM"))

            # ---- constants (zT in chunks so A1 starts after the first one) ----
            zTc = []
            for c in range(NL):
                zt = const.tile([128, P], F16, name=f"zT_sb{c}")
                nc.sync.dma_start(zt[:], zT[:, c * P : (c + 1) * P])
                zTc.append(zt)
            W0_sb = const.tile([128, H], F16)
            nc.sync.dma_start(W0_sb[:], W0f[:])
            W1_sb = [const.tile([128, H], F16, name=f"W1_sb{i}") for i in range(2)]
            LW_sb = [const.tile([128, XF], F16, name=f"LW_sb{i}") for i in range(2)]
            b0_sb = [const.tile([128, 1], F32, name=f"b0_sb{i}") for i in range(2)]
            b1_sb = [const.tile([128, 1], F32, name=f"b1_sb{i}") for i in range(2)]
            for i in range(2):
                nc.sync.dma_start(W1_sb[i][:], W1f[i])
                nc.sync.dma_start(LW_sb[i][:], LWf[i])
                nc.sync.dma_start(b0_sb[i][:], b0c[i])
                nc.sync.dma_start(b1_sb[i][:], b1c[i])
            lbb_sb = const.tile([128, XF], F32)
            nc.sync.dma_start(lbb_sb[:], lbb[:])

            # collective buffers: u1 allgathered in 4 chunks of 2 node-blocks
            cin = [dram.tile([H, H], F16, name=f"cin{j}") for j in range(4)]
            cout = [
                dram.tile([N_CORES * H, H], F16, addr_space="Shared", name=f"cout{j}")
                for j in range(4)
            ]

            # ---- A1: u0 = z @ W0 as f16 [node-part, h-free] ----
            u0 = []
            for i in range(SC):
                pa = psa.tile([128, H], F32, tag="pa", name=f"pa1_{i}")
                nc.tensor.matmul(
                    pa[:],
                    zTc[i // NL][:, (i % NL) * 128 : (i % NL + 1) * 128],
                    W0_sb[:],
                    start=True,
                    stop=True,
                )
                u = acts.tile([128, H], F16, tag="u0", bufs=SC, name=f"u0_{i}")
                nc.vector.tensor_copy(u[:], pa[:])
                u0.append(u)

            # ---- B1: y1^T[h, d] = sum_s u0[s, h] * Bt[s, d] ----
            # Two sweeps: B1a covers local dst columns 0:512 (whose Bt halves
            # stay SBUF-resident for reuse in conv2), then B1b covers 512:1024.
            # Finishing half of h1 early lets the first AllGathers launch
            # ~35us sooner, hidden behind B1b's matmuls.
            ps1 = [
                [
                    psb.tile([128, 512], F32, tag="pb", name=f"pb1_{hc}_{dc}")
                    for dc in range(2)
                ]
                for hc in range(2)
            ]
            h1 = [
                [
                    acts.tile([128, 512], F16, tag="h1", bufs=4, name=f"h1_{hc}_{dc}")
                    for dc in range(2)
                ]
                for hc in range(2)
            ]
            bt0 = []
            for sc in range(SC):
                b0t = bres.tile([128, 512], F16, tag="bt0", bufs=SC, name=f"bt0_{sc}")
                nc.sync.dma_start(b0t[:], Bt[sc * 128 : (sc + 1) * 128, 0:512])
                bt0.append(b0t)
                for hc in range(2):
                    nc.tensor.matmul(
                        ps1[hc][0][:],
                        u0[sc][:, hc * 128 : (hc + 1) * 128],
                        b0t[:],
                        start=(sc == 0),
                        stop=(sc == SC - 1),
                    )

            def a2_chunk(jp):
                # u1[n, g] = sum_h h1[n, h] W1[h, g] for 2 local node blocks,
                # then allgather them (chunk jp).
                for j in (2 * jp, 2 * jp + 1):
                    pa = psa.tile([128, H], F32, tag="pa", name=f"pa2_{j}")
                    for hc in range(2):
                        nc.tensor.matmul(
                            pa[:],
                            h1[hc][j // 4][:, (j % 4) * 128 : (j % 4 + 1) * 128],
                            W1_sb[hc][:],
                            start=(hc == 0),
                            stop=(hc == 1),
                        )
                    u = acts.tile([128, H], F16, tag="u1", bufs=NL, name=f"u1_{j}")
                    nc.vector.tensor_copy(u[:], pa[:])
                    nc.sync.dma_start(
                        cin[jp][(j % 2) * 128 : (j % 2 + 1) * 128, :], u[:]
                    )
                nc.gpsimd.collective_compute(
                    "AllGather",
                    mybir.AluOpType.bypass,
                    ins=[cin[jp].opt()],
                    outs=[cout[jp].opt()],
                    replica_groups=[list(range(N_CORES))],
                )

            for hc in range(2):
                nc.scalar.activation(
                    h1[hc][0][:], ps1[hc][0][:], relu, bias=b0_sb[hc][:], scale=1.0
                )
            a2_chunk(0)
            a2_chunk(1)

            for sc in range(SC):
                b1t = bstr.tile([128, 512], F16, tag="bts", name=f"bt1_{sc}")
                nc.gpsimd.dma_start(b1t[:], Bt[sc * 128 : (sc + 1) * 128, 512:1024])
                for hc in range(2):
                    nc.tensor.matmul(
                        ps1[hc][1][:],
                        u0[sc][:, hc * 128 : (hc + 1) * 128],
                        b1t[:],
                        start=(sc == 0),
                        stop=(sc == SC - 1),
                    )

            for hc in range(2):
                nc.scalar.activation(
                    h1[hc][1][:], ps1[hc][1][:], relu, bias=b0_sb[hc][:], scale=1.0
                )
            a2_chunk(2)
            a2_chunk(3)

            # ---- B2: y2^T[g, d] = sum_s u1[s, g] * Bt[s, d] ----
            # consume allgathered u1 chunks in arrival order
            ps2 = [
                [
                    psb.tile([128, 512], F32, tag="pb", name=f"pb2_{gc}_{dc}")
                    for dc in range(2)
                ]
                for gc in range(2)
            ]
            first = True
            n_done = 0
            for jp in range(4):
                for r in range(N_CORES):
                    for j in (2 * jp, 2 * jp + 1):
                        sc = r * NL + j
                        ug = acts.tile(
                            [128, H], F16, tag="ug", bufs=16, name=f"ug_{sc}"
                        )
                        nc.scalar.dma_start(
                            ug[:],
                            cout[jp][
                                r * H + (j % 2) * 128 : r * H + (j % 2 + 1) * 128, :
                            ],
                        )
                        b1t = bstr.tile([128, 512], F16, tag="bts", name=f"bt2_{sc}")
                        eng = nc.sync if sc % 2 == 0 else nc.gpsimd
                        eng.dma_start(
                            b1t[:], Bt[sc * 128 : (sc + 1) * 128, 512:1024]
                        )
                        n_done += 1
                        last = n_done == SC
                        for gc in range(2):
                            nc.tensor.matmul(
                                ps2[gc][0][:],
                                ug[:, gc * 128 : (gc + 1) * 128],
                                bt0[sc][:],
                                start=first,
                                stop=last,
                            )
                            nc.tensor.matmul(
                                ps2[gc][1][:],
                                ug[:, gc * 128 : (gc + 1) * 128],
                                b1t[:],
                                start=first,
                                stop=last,
                            )
                        first = False

            h2 = [
                [
                    acts.tile([128, 512], F16, tag="h2", bufs=4, name=f"h2_{gc}_{dc}")
                    for dc in range(2)
                ]
                for gc in range(2)
            ]
            for gc in range(2):
                for dc in range(2):
                    nc.scalar.activation(
                        h2[gc][dc][:],
                        ps2[gc][dc][:],
                        relu,
                        bias=b1_sb[gc][:],
                        scale=1.0,
                    )

            # ---- final linear: out[n, o] = h2[n, :] @ lin_W + lin_b ----
            for lc in range(NL):
                pf = psa.tile([128, XF], F32, tag="pa", name=f"pf_{lc}")
                for gc in range(2):
                    nc.tensor.matmul(
                        pf[:],
                        h2[gc][lc // 4][:, (lc % 4) * 128 : (lc % 4 + 1) * 128],
                        LW_sb[gc][:],
                        start=(gc == 0),
                        stop=(gc == 1),
                    )
                ot = acts.tile([128, XF], F32, tag="ot", bufs=2, name=f"ot_{lc}")
                nc.vector.tensor_add(ot[:], pf[:], lbb_sb[:])
                nc.sync.dma_start(OUT[lc * 128 : (lc + 1) * 128, :], ot[:])

    nc.compile()
    return nc


def _get_module():
    if "nc" not in _STATE:
        _STATE["nc"] = _build_module()
    return _STATE["nc"]


def kernel(z_, edge_index, edge_attr, W0, b0, W1, b1, lin_W, lin_b):
    global LAST_RESULT
    z_ = np.asarray(z_, dtype=np.float32)
    edge_index = np.asarray(edge_index)
    edge_attr = np.asarray(edge_attr, dtype=np.float32)
    W0 = np.asarray(W0, dtype=np.float32)
    b0 = np.asarray(b0, dtype=np.float32)
    W1 = np.asarray(W1, dtype=np.float32)
    b1 = np.asarray(b1, dtype=np.float32)
    lin_W = np.asarray(lin_W, dtype=np.float32)
    lin_b = np.asarray(lin_b, dtype=np.float32)

    src = edge_index[0].astype(np.int64)
    dst = edge_index[1].astype(np.int64)

    # deg = segment_sum(w, dst) over edges + unit self-loops
    deg = np.zeros(N, np.float32)
    np.add.at(deg, dst, edge_attr)
    deg += 1.0
    s = (1.0 / np.sqrt(deg)).astype(np.float32)

    # Bt[s, d] = s[s] * (A_w + I)^T * s[d]
    Bm = np.zeros((N, N), np.float32)
    np.add.at(Bm, (src, dst), edge_attr)
    Bm[np.arange(N), np.arange(N)] += 1.0
    Bm *= s[:, None]
    Bm *= s[None, :]
    Bh = Bm.astype(np.float16)
    del Bm

    zT16 = np.ascontiguousarray(z_.T).astype(np.float16)
    common = {
        "zT": zT16,
        "W0f": W0.astype(np.float16),
        "W1f": np.ascontiguousarray(W1.reshape(2, 128, H)).astype(np.float16),
        "LWf": np.ascontiguousarray(lin_W.reshape(2, 128, XF)).astype(np.float16),
        "b0c": np.ascontiguousarray(b0.reshape(2, 128, 1)),
        "b1c": np.ascontiguousarray(b1.reshape(2, 128, 1)),
        "lbb": np.ascontiguousarray(np.broadcast_to(lin_b, (128, XF))),
    }
    in_maps = [
        dict(common, Bt=np.ascontiguousarray(Bh[:, k * P : (k + 1) * P]))
        for k in range(N_CORES)
    ]

    nc = _get_module()
    res = run_bass_kernel_spmd(
        nc, in_maps, core_ids=list(range(N_CORES)), trace=TRACE
    )
    LAST_RESULT = res
    out = np.concatenate([res.results[k]["OUT"] for k in range(N_CORES)], axis=0)

    # new_edge_index: static full row-major grid, like the reference's
    # repeat/tile of arange(n) (int32 under default jax, int64 under x64).
    idt = np.int64 if edge_index.dtype == np.int64 else np.int32
    nei = np.empty((2, N * N), dtype=idt)
    nei[0] = np.repeat(np.arange(N, dtype=idt), N)
    nei[1] = np.tile(np.arange(N, dtype=idt), N)
    return out, nei


# revision 7
# speedup vs baseline: 1.1864x; 1.0524x over previous
"""GCN decoder (2x GCNConv + linear) on 8 Trainium2 NeuronCores.

Math (from the reference):
  adj = sigmoid(z z^T) values are discarded; only new_edge_index (a static
  full row-major grid of indices) survives. The two GCN convs run on the
  ORIGINAL sparse edges + self-loops:
      conv(x) = relu(D^-1/2 (A_w + I) D^-1/2 (x @ W) + b)
  then out = h @ lin_W + lin_b.

Strategy:
  Densify the normalized adjacency on the host:
      Bt[s, d] = s[s] * (A_w + I)^T[s, d] * s[d],   s = deg^-1/2
  so each conv aggregation is a dense matmul y^T = u_stat @ Bt (node
  contraction on the PE partition axis). Aggregation outputs live in
  transposed [feature, node] layout, which each following x @ W step
  consumes directly as the stationary operand — no on-device transposes.

  Sharding: column-shard Bt across the 8 cores (each core owns 1024
  destination nodes); replicate z and the small weights. Between the convs
  each core computes u1 = h1 @ W1 for ITS OWN nodes only (u1 is row-wise,
  so it needs no remote data), and u1 is AllGathered in 4 chunks; conv2's
  accumulation consumes the chunks in arrival order, hiding the collective
  behind matmuls. The dc=0 half of Bt stays resident in SBUF after conv1
  so conv2 only streams the other half. Adjacency and activations are f16
  (~5e-4 relative error) halving HBM traffic; PSUM accumulates in f32.
"""

from contextlib import ExitStack

import numpy as np

import concourse.bass as bass
import concourse.tile as tile
from concourse import bacc, mybir
from concourse.bass_utils import run_bass_kernel_spmd

N = 8192  # nodes
ZF = 128  # z features
H = 256  # GCN hidden width
XF = 128  # output features
N_CORES = 8
P = N // N_CORES  # 1024 destination nodes per core
SC = N // 128  # 64 source-node chunks
NL = P // 128  # 8 local node chunks
F16 = mybir.dt.float16
F32 = mybir.dt.float32

# Set by a test harness to capture an NTFF profile of the SPMD run.
TRACE = False
LAST_RESULT = None

_STATE = {}


def _build_module():
    nc = bacc.Bacc("TRN2", target_bir_lowering=False, debug=False, num_devices=N_CORES)

    zT = nc.dram_tensor("zT", [ZF, N], F16, kind="ExternalInput").ap()
    Bt = nc.dram_tensor("Bt", [N, P], F16, kind="ExternalInput").ap()
    W0f = nc.dram_tensor("W0f", [ZF, H], F16, kind="ExternalInput").ap()
    W1f = nc.dram_tensor("W1f", [2, 128, H], F16, kind="ExternalInput").ap()
    LWf = nc.dram_tensor("LWf", [2, 128, XF], F16, kind="ExternalInput").ap()
    b0c = nc.dram_tensor("b0c", [2, 128, 1], F32, kind="ExternalInput").ap()
    b1c = nc.dram_tensor("b1c", [2, 128, 1], F32, kind="ExternalInput").ap()
    lbb = nc.dram_tensor("lbb", [128, XF], F32, kind="ExternalInput").ap()
    OUT = nc.dram_tensor("OUT", [P, XF], F32, kind="ExternalOutput").ap()

    relu = mybir.ActivationFunctionType.Relu

    with tile.TileContext(nc) as tc:
        with ExitStack() as ctx:
            const = ctx.enter_context(tc.tile_pool(name="const", bufs=1))
            acts = ctx.enter_context(tc.tile_pool(name="acts", bufs=1))
            bres = ctx.enter_context(tc.tile_pool(name="bres", bufs=1))
            bstr = ctx.enter_context(tc.tile_pool(name="bstr", bufs=16))
            psa = ctx.enter_context(tc.tile_pool(name="psa", bufs=4, space="PSUM"))
            psb = ctx.enter_context(tc.tile_pool(name="psb", bufs=4, space="PSUM"))
            dram = ctx.enter_context(tc.tile_pool(name="dram", bufs=1, space="DRAM"))

            # ---- constants (zT in chunks so A1 starts after the first one) ----
            zTc = []
            zt_engs = [nc.sync, nc.scalar]
            for c in range(NL):
                zt = const.tile([128, P], F16, name=f"zT_sb{c}")
                zt_engs[c % 2].dma_start(zt[:], zT[:, c * P : (c + 1) * P])
                zTc.append(zt)
            W0_sb = const.tile([128, H], F16)
            nc.sync.dma_start(W0_sb[:], W0f[:])
            W1_sb = [const.tile([128, H], F16, name=f"W1_sb{i}") for i in range(2)]
            LW_sb = [const.tile([128, XF], F16, name=f"LW_sb{i}") for i in range(2)]
            b0_sb = [const.tile([128, 1], F32, name=f"b0_sb{i}") for i in range(2)]
            b1_sb = [const.tile([128, 1], F32, name=f"b1_sb{i}") for i in range(2)]
            for i in range(2):
                nc.sync.dma_start(W1_sb[i][:], W1f[i])
                nc.sync.dma_start(LW_sb[i][:], LWf[i])
                nc.sync.dma_start(b0_sb[i][:], b0c[i])
                nc.sync.dma_start(b1_sb[i][:], b1c[i])
            lbb_sb = const.tile([128, XF], F32)
            nc.sync.dma_start(lbb_sb[:], lbb[:])

            # collective buffers: u1 allgathered in 4 chunks of 2 node-blocks
            cin = [dram.tile([2 * H, H], F16, name=f"cin{j}") for j in range(2)]
            cout = [
                dram.tile([N_CORES * 2 * H, H], F16, addr_space="Shared", name=f"cout{j}")
                for j in range(2)
            ]

            # ---- A1: u0 = z @ W0 as f16 [node-part, h-free] ----
            u0 = []
            for i in range(SC):
                pa = psa.tile([128, H], F32, tag="pa", name=f"pa1_{i}")
                nc.tensor.matmul(
                    pa[:],
                    zTc[i // NL][:, (i % NL) * 128 : (i % NL + 1) * 128],
                    W0_sb[:],
                    start=True,
                    stop=True,
                )
                u = acts.tile([128, H], F16, tag="u0", bufs=SC, name=f"u0_{i}")
                nc.vector.tensor_copy(u[:], pa[:])
                u0.append(u)

            # ---- B1: y1^T[h, d] = sum_s u0[s, h] * Bt[s, d] ----
            # Two sweeps: B1a covers local dst columns 0:512 (whose Bt halves
            # stay SBUF-resident for reuse in conv2), then B1b covers 512:1024.
            # Finishing half of h1 early lets the first AllGathers launch
            # ~35us sooner, hidden behind B1b's matmuls.
            ps1 = [
                [
                    psb.tile([128, 512], F32, tag="pb", name=f"pb1_{hc}_{dc}")
                    for dc in range(2)
                ]
                for hc in range(2)
            ]
            h1 = [
                [
                    acts.tile([128, 512], F16, tag="h1", bufs=4, name=f"h1_{hc}_{dc}")
                    for dc in range(2)
                ]
                for hc in range(2)
            ]
            bt0 = []
            for sc in range(SC):
                b0t = bres.tile([128, 512], F16, tag="bt0", bufs=SC, name=f"bt0_{sc}")
                nc.sync.dma_start(b0t[:], Bt[sc * 128 : (sc + 1) * 128, 0:512])
                bt0.append(b0t)
                for hc in range(2):
                    nc.tensor.matmul(
                        ps1[hc][0][:],
                        u0[sc][:, hc * 128 : (hc + 1) * 128],
                        b0t[:],
                        start=(sc == 0),
                        stop=(sc == SC - 1),
                    )

            def a2_chunk(hp):
                # u1[n, g] = sum_h h1[n, h] W1[h, g] for 4 local node blocks,
                # then allgather them (half hp).
                for j in range(4 * hp, 4 * hp + 4):
                    pa = psa.tile([128, H], F32, tag="pa", name=f"pa2_{j}")
                    for hc in range(2):
                        nc.tensor.matmul(
                            pa[:],
                            h1[hc][j // 4][:, (j % 4) * 128 : (j % 4 + 1) * 128],
                            W1_sb[hc][:],
                            start=(hc == 0),
                            stop=(hc == 1),
                        )
                    u = acts.tile([128, H], F16, tag="u1", bufs=NL, name=f"u1_{j}")
                    nc.vector.tensor_copy(u[:], pa[:])
                    nc.sync.dma_start(
                        cin[hp][(j % 4) * 128 : (j % 4 + 1) * 128, :], u[:]
                    )
                nc.gpsimd.collective_compute(
                    "AllGather",
                    mybir.AluOpType.bypass,
                    ins=[cin[hp].opt()],
                    outs=[cout[hp].opt()],
                    replica_groups=[list(range(N_CORES))],
                )

            for hc in range(2):
                nc.scalar.activation(
                    h1[hc][0][:], ps1[hc][0][:], relu, bias=b0_sb[hc][:], scale=1.0
                )
            a2_chunk(0)

            for sc in range(SC):
                b1t = bstr.tile([128, 512], F16, tag="bts", name=f"bt1_{sc}")
                nc.scalar.dma_start(b1t[:], Bt[sc * 128 : (sc + 1) * 128, 512:1024])
                for hc in range(2):
                    nc.tensor.matmul(
                        ps1[hc][1][:],
                        u0[sc][:, hc * 128 : (hc + 1) * 128],
                        b1t[:],
                        start=(sc == 0),
                        stop=(sc == SC - 1),
                    )

            for hc in range(2):
                nc.scalar.activation(
                    h1[hc][1][:], ps1[hc][1][:], relu, bias=b0_sb[hc][:], scale=1.0
                )
            a2_chunk(1)

            # ---- B2: y2^T[g, d] = sum_s u1[s, g] * Bt[s, d] ----
            # consume allgathered u1 chunks in arrival order
            ps2 = [
                [
                    psb.tile([128, 512], F32, tag="pb", name=f"pb2_{gc}_{dc}")
                    for dc in range(2)
                ]
                for gc in range(2)
            ]
            first = True
            n_done = 0
            for hp in range(2):
                for r in range(N_CORES):
                    for j in range(4 * hp, 4 * hp + 4):
                        sc = r * NL + j
                        ug = acts.tile(
                            [128, H], F16, tag="ug", bufs=16, name=f"ug_{sc}"
                        )
                        nc.scalar.dma_start(
                            ug[:],
                            cout[hp][
                                r * 2 * H
                                + (j % 4) * 128 : r * 2 * H
                                + (j % 4 + 1) * 128,
                                :,
                            ],
                        )
                        b1t = bstr.tile([128, 512], F16, tag="bts", name=f"bt2_{sc}")
                        eng = nc.sync
                        eng.dma_start(
                            b1t[:], Bt[sc * 128 : (sc + 1) * 128, 512:1024]
                        )
                        n_done += 1
                        last = n_done == SC
                        for gc in range(2):
                            nc.tensor.matmul(
                                ps2[gc][0][:],
                                ug[:, gc * 128 : (gc + 1) * 128],
                                bt0[sc][:],
                                start=first,
                                stop=last,
                            )
                            nc.tensor.matmul(
                                ps2[gc][1][:],
                                ug[:, gc * 128 : (gc + 1) * 128],
                                b1t[:],
                                start=first,
                                stop=last,
                            )
                        first = False

            h2 = [
                [
                    acts.tile([128, 512], F16, tag="h2", bufs=4, name=f"h2_{gc}_{dc}")
                    for dc in range(2)
                ]
                for gc in range(2)
            ]
            for gc in range(2):
                for dc in range(2):
                    nc.scalar.activation(
                        h2[gc][dc][:],
                        ps2[gc][dc][:],
                        relu,
                        bias=b1_sb[gc][:],
                        scale=1.0,
                    )

            # ---- final linear: out[n, o] = h2[n, :] @ lin_W + lin_b ----
            for lc in range(NL):
                pf = psa.tile([128, XF], F32, tag="pa", name=f"pf_{lc}")
                for gc in range(2):
                    nc.tensor.matmul(
                        pf[:],
                        h2[gc][lc // 4][:, (lc % 4) * 128 : (lc % 4 + 1) * 128],
                        LW_sb[gc][:],
                        start=(gc == 0),
                        stop=(gc == 1),
                    )
                ot = acts.tile([128, XF], F32, tag="ot", bufs=2, name=f"ot_{lc}")
                nc.vector.tensor_add(ot[:], pf[:], lbb_sb[:])
                nc.sync.dma_start(OUT[lc * 128 : (lc + 1) * 128, :], ot[:])

    nc.compile()
    return nc


def _get_module():
    if "nc" not in _STATE:
        _STATE["nc"] = _build_module()
    return _STATE["nc"]


def kernel(z_, edge_index, edge_attr, W0, b0, W1, b1, lin_W, lin_b):
    global LAST_RESULT
    z_ = np.asarray(z_, dtype=np.float32)
    edge_index = np.asarray(edge_index)
    edge_attr = np.asarray(edge_attr, dtype=np.float32)
    W0 = np.asarray(W0, dtype=np.float32)
    b0 = np.asarray(b0, dtype=np.float32)
    W1 = np.asarray(W1, dtype=np.float32)
    b1 = np.asarray(b1, dtype=np.float32)
    lin_W = np.asarray(lin_W, dtype=np.float32)
    lin_b = np.asarray(lin_b, dtype=np.float32)

    src = edge_index[0].astype(np.int64)
    dst = edge_index[1].astype(np.int64)

    # deg = segment_sum(w, dst) over edges + unit self-loops
    deg = np.zeros(N, np.float32)
    np.add.at(deg, dst, edge_attr)
    deg += 1.0
    s = (1.0 / np.sqrt(deg)).astype(np.float32)

    # Bt[s, d] = s[s] * (A_w + I)^T * s[d]
    Bm = np.zeros((N, N), np.float32)
    np.add.at(Bm, (src, dst), edge_attr)
    Bm[np.arange(N), np.arange(N)] += 1.0
    Bm *= s[:, None]
    Bm *= s[None, :]
    Bh = Bm.astype(np.float16)
    del Bm

    zT16 = np.ascontiguousarray(z_.T).astype(np.float16)
    common = {
        "zT": zT16,
        "W0f": W0.astype(np.float16),
        "W1f": np.ascontiguousarray(W1.reshape(2, 128, H)).astype(np.float16),
        "LWf": np.ascontiguousarray(lin_W.reshape(2, 128, XF)).astype(np.float16),
        "b0c": np.ascontiguousarray(b0.reshape(2, 128, 1)),
        "b1c": np.ascontiguousarray(b1.reshape(2, 128, 1)),
        "lbb": np.ascontiguousarray(np.broadcast_to(lin_b, (128, XF))),
    }
    in_maps = [
        dict(common, Bt=np.ascontiguousarray(Bh[:, k * P : (k + 1) * P]))
        for k in range(N_CORES)
    ]

    nc = _get_module()
    res = run_bass_kernel_spmd(
        nc, in_maps, core_ids=list(range(N_CORES)), trace=TRACE
    )
    LAST_RESULT = res
    out = np.concatenate([res.results[k]["OUT"] for k in range(N_CORES)], axis=0)

    # new_edge_index: static full row-major grid, like the reference's
    # repeat/tile of arange(n) (int32 under default jax, int64 under x64).
    idt = np.int64 if edge_index.dtype == np.int64 else np.int32
    nei = np.empty((2, N * N), dtype=idt)
    nei[0] = np.repeat(np.arange(N, dtype=idt), N)
    nei[1] = np.tile(np.arange(N, dtype=idt), N)
    return out, nei
